# revision 7
# baseline (speedup 1.0000x reference)
"""Bass/Trainium2 kernel for nn_BayesConv2dMF (per-sample-weight 3x3 conv).

Contract: kernel(**inputs) takes FULL unsharded inputs
  input      [32, 128, 56, 56] f32
  eps        [32, 128, 128, 3, 3] f32
  weight_psi [128, 128, 3, 3] f32
  weight_mu  [128, 128, 3, 3] f32
and returns the FULL output [32, 128, 56, 56] f32.

Strategy: data-parallel over batch across 8 NeuronCores (4 images/core).
Host prep: psi/mu are fed pre-transposed as [CI, K9, CO] f32 so the device
can DMA them in per-tap-group slices (the natural [CO,CI,3,3] layout has
taps innermost, which makes group slices 6-byte-granular). All math
(exp, eps*exp(psi)+mu, conv) stays on device.

Per image on-core (software-pipelined one image ahead):
  eps -> SBUF via SWDGE cast-DMA f32->bf16 (natural [CO, CI*9] layout)
  per tap: transpose eps -> [CI, k, CO]. Image 0 (ramp-critical) uses PE
      transposes pipelined per tap-group; steady-state images use the DMA
      XBAR transpose (dma_start_transpose, spread over the SP/Act/DVE
      rings) so the PE does nothing but conv matmuls.
  DVE: wT = epsT * exp(psiT) then wT += muT  -> [CI, K9, CO] bf16
  x   -> plain [CI, 56, 56] bf16 tile via SWDGE cast-DMA (fully contiguous
      per partition -> full-rate; no padding needed at all)
  conv: chunks of 7 output rows; taps outer so one weight load feeds the
      live chunks; 9 PSUM-accumulating matmuls per chunk (K=CI=128, bf16).
      Edge handling: tap (1,1) goes first with start=True over the full
      chunk; kw!=1 taps restrict output columns and kh!=1 taps restrict
      rows on the top/bottom chunks (out-of-range x reads are implicit
      zeros that are simply never accumulated).
  PSUM -> SBUF bf16 (ScalarE) -> DRAM bf16 (SP HWDGE); host upcasts.
  A HAM warm-up burst keeps the PE clock ungated through the input ramp.
"""

import numpy as np

import concourse.bass as bass
import concourse.tile as tile
from concourse import bacc, mybir
from concourse.bass_utils import run_bass_kernel_spmd
from concourse.masks import make_identity

B, CO, CI, KH, KW, H, W = 32, 128, 128, 3, 3, 56, 56
K9 = KH * KW
N_CORES = 8
BPC = B // N_CORES  # images per core
RB = 7  # output rows per PSUM chunk
NCHUNK = H // RB  # 8 chunks per image
F32 = mybir.dt.float32
BF16 = mybir.dt.bfloat16

# tap-group order: g1 (taps 3,4,5) first so tap 4 = (kh=1,kw=1) leads
GROUPS = [1, 0, 2]
# within-chunk tap order: group g1 first, tap (kh,1) first inside each group
TAP_ORDER = [4, 3, 5, 1, 0, 2, 7, 6, 8]

N_WARM = 26  # HAM warm-up matmuls (must fit in the pre-conv PE idle window)

NPSO = 5  # rolling PSUM chunk slots (PSUM is 8 banks: 5 + 2 pswt + 1 warm)

# image-0 x row pieces (prefix loads so early conv parts can start)
X0_BOUNDS = [0, 22, 43, H]


def tap_ranges(k):
    """Output-column range and x-column range for tap k (W-edge handling)."""
    kh, kw = divmod(k, KW)
    if kw == 0:
        return kh, 1, W, 0, W - 1  # out cols 1..55 <- x cols 0..54
    if kw == 2:
        return kh, 0, W - 1, 1, W  # out cols 0..54 <- x cols 1..55
    return kh, 0, W, 0, W  # full


def emit(nc, tc, ctx, x_d, eps_d, psit_d, mut_d, out_d):
    const = ctx.enter_context(tc.tile_pool(name="const", bufs=1))
    wpool = ctx.enter_context(tc.tile_pool(name="wpool", bufs=2))
    opool = ctx.enter_context(tc.tile_pool(name="opool", bufs=2))
    psw = ctx.enter_context(tc.tile_pool(name="psw", bufs=1, space="PSUM"))
    pso = ctx.enter_context(tc.tile_pool(name="pso", bufs=1, space="PSUM"))

    ident = const.tile([128, 128], BF16)
    make_identity(nc, ident)

    # HAM warm-up: dummy matmuls fill the pre-conv PE idle window so the
    # activity monitor releases the clock gate before the real stream.
    warm_ps = psw.tile([128, 64], F32, tag="warm", name="warm_ps", bufs=1)
    for _ in range(N_WARM):
        nc.tensor.matmul(warm_ps, ident, ident[:, :64], start=True, stop=True)

    # shared weights, host-pre-transposed to [CI, K9, CO]
    psi_t = const.tile([CI, K9, CO], F32)
    mu_t = const.tile([CI, K9, CO], F32)
    exp_psi = const.tile([CI, K9, CO], BF16)
    mu_bf = const.tile([CI, K9, CO], BF16)
    for g in GROUPS:
        sl = slice(3 * g, 3 * g + 3)
        nc.sync.dma_start(psi_t[:, sl, :], psit_d[:, sl, :])
        nc.sync.dma_start(mu_t[:, sl, :], mut_d[:, sl, :])
    for g in GROUPS:
        sl = slice(3 * g, 3 * g + 3)
        nc.scalar.activation(
            exp_psi[:, sl, :], psi_t[:, sl, :], mybir.ActivationFunctionType.Exp
        )
    # one-time bf16 cast of muT for the steady-state DVE adds (image 0 uses
    # the f32 muT directly so its chain doesn't wait for this)
    nc.scalar.copy(mu_bf, mu_t)

    NXP = 3
    xts = [const.tile([CI, H, W], BF16, name=f"xt{i}", tag=f"xt{i}") for i in range(NXP)]

    wTs = {}
    out_sbs = {}

    def prep(b):
        # per-sample weights: cast-DMA eps (bf16), transpose each tap, then
        # DVE: wT = epsT * exp(psiT) + muT.
        eps_t = wpool.tile([CO, CI, K9], BF16, tag="eps", name=f"eps{b}")
        nc.gpsimd.dma_start(
            eps_t, eps_d[b].rearrange("co ci kh kw -> co ci (kh kw)")
        )
        wT = wpool.tile([CI, K9, CO], BF16, tag="wT", name=f"wT{b}")
        if b == 0:
            # ramp-critical: PE transposes, pipelined per 3-tap group
            psum_wt = psw.tile([CI, K9, CO], BF16, tag="pswt", name="pswt0", bufs=1)
            for g in GROUPS:
                sl = slice(3 * g, 3 * g + 3)
                for k in range(3 * g, 3 * g + 3):
                    nc.tensor.transpose(psum_wt[:, k, :], eps_t[:, :, k], ident)
                nc.vector.tensor_mul(
                    wT[:, sl, :], psum_wt[:, sl, :], exp_psi[:, sl, :]
                )
                nc.vector.tensor_add(wT[:, sl, :], wT[:, sl, :], mu_t[:, sl, :])
        else:
            # steady state: DMA XBAR transposes keep the PE conv-only.
            # The XBAR needs a contiguous input row, so first repack
            # [CO, CI, K9] -> [CO, K9, CI] on DVE (free-dim permutation),
            # then transpose per tap, spread over both HWDGE rings.
            eps_r = wpool.tile([CO, K9, CI], BF16, tag="epsr", name=f"epsr{b}")
            nc.vector.tensor_copy(
                eps_r, eps_t.rearrange("co ci k -> co k ci")
            )
            epsT = wpool.tile([CI, K9, CO], BF16, tag="epsT", name=f"epsT{b}")
            for k, eng in zip(range(K9), [nc.sync, nc.scalar] * 5):
                eng.dma_start_transpose(epsT[:, k, :], eps_r[:, k, :])
            nc.vector.tensor_mul(wT, epsT, exp_psi)
            nc.vector.tensor_add(wT, wT, mu_bf)
        wTs[b] = wT

        # input image: SWDGE cast-DMA, fully contiguous per partition
        xt = xts[b % NXP]
        bounds = X0_BOUNDS if b == 0 else [0, H]
        for lo, hi in zip(bounds[:-1], bounds[1:]):
            nc.gpsimd.dma_start(xt[:, lo:hi, :], x_d[b][:, lo:hi, :])
        out_sbs[b] = opool.tile([CO, H, W], BF16, tag="osb", name=f"osb{b}")

    slot_counter = [0]

    def conv_part(b, r0, nch):
        xt = xts[b % NXP]
        wT = wTs[b]
        out_sb = out_sbs[b]
        rows = nch * RB
        pss = []
        for c in range(nch):
            s = slot_counter[0] % NPSO
            slot_counter[0] += 1
            ps = pso.tile([CO, RB, W], F32, tag=f"pso{s}", name=f"ps_{b}_{r0}_{c}")
            pss.append(ps)
        # taps outer: one weight load per tap feeds all live chunk matmuls
        for i, k in enumerate(TAP_ORDER):
            kh, olo, ohi, xlo, xhi = tap_ranges(k)
            for c in range(nch):
                arow = r0 + c * RB
                rlo = 1 if (arow == 0 and kh == 0) else 0
                rhi = RB - (1 if (arow + RB == H and kh == 2) else 0)
                xr = arow + rlo + kh - 1
                nc.tensor.matmul(
                    pss[c][:, rlo:rhi, olo:ohi],
                    wT[:, k, :],
                    xt[:, xr : xr + rhi - rlo, xlo:xhi],
                    start=(i == 0),
                    stop=(i == K9 - 1),
                )
        for c in range(nch):
            dst = out_sb[:, r0 + c * RB : r0 + (c + 1) * RB, :]
            nc.scalar.copy(dst, pss[c])
        nc.sync.dma_start(
            out_d[b][:, r0 : r0 + rows, :], out_sb[:, r0 : r0 + rows, :]
        )

    # software-pipelined emission: image b+1's weight/x prep is emitted
    # between the conv parts of image b so its DVE/DMA work interleaves.
    # The final image tapers to 1-chunk parts so the drain tail shrinks.
    prep(0)
    for b in range(BPC):
        if b == 0:
            conv_part(b, 0, 3)
            conv_part(b, 3 * RB, 3)
            prep(1)
            conv_part(b, 6 * RB, 2)
        elif b + 1 < BPC:
            conv_part(b, 0, 2)
            conv_part(b, 2 * RB, 2)
            prep(b + 1)
            conv_part(b, 4 * RB, 2)
            conv_part(b, 6 * RB, 2)
        else:
            conv_part(b, 0, 2)
            conv_part(b, 2 * RB, 2)
            conv_part(b, 4 * RB, 2)
            conv_part(b, 6 * RB, 1)
            conv_part(b, 7 * RB, 1)


def build():
    from contextlib import ExitStack

    nc = bacc.Bacc("TRN2", target_bir_lowering=False, debug=False, num_devices=N_CORES)
    x_d = nc.dram_tensor("input", [BPC, CI, H, W], F32, kind="ExternalInput").ap()
    eps_d = nc.dram_tensor(
        "eps", [BPC, CO, CI, KH, KW], F32, kind="ExternalInput"
    ).ap()
    psit_d = nc.dram_tensor("psi_t", [CI, K9, CO], F32, kind="ExternalInput").ap()
    mut_d = nc.dram_tensor("mu_t", [CI, K9, CO], F32, kind="ExternalInput").ap()
    out_d = nc.dram_tensor("out", [BPC, CO, H, W], BF16, kind="ExternalOutput").ap()

    with tile.TileContext(nc) as tc:
        with ExitStack() as ctx:
            emit(nc, tc, ctx, x_d, eps_d, psit_d, mut_d, out_d)
    nc.compile()
    return nc


_NC_CACHE = None


def kernel(input, eps, weight_psi, weight_mu, **run_kwargs):
    global _NC_CACHE
    if _NC_CACHE is None:
        _NC_CACHE = build()
    nc = _NC_CACHE
    # host layout prep of the replicated weights: [CO,CI,KH,KW] -> [CI,K9,CO]
    psi_t = np.ascontiguousarray(
        weight_psi.reshape(CO, CI, K9).transpose(1, 2, 0), dtype=np.float32
    )
    mu_t = np.ascontiguousarray(
        weight_mu.reshape(CO, CI, K9).transpose(1, 2, 0), dtype=np.float32
    )
    in_maps = []
    for c in range(N_CORES):
        sl = slice(c * BPC, (c + 1) * BPC)
        in_maps.append(
            {
                "input": np.ascontiguousarray(input[sl], dtype=np.float32),
                "eps": np.ascontiguousarray(eps[sl], dtype=np.float32),
                "psi_t": psi_t,
                "mu_t": mu_t,
            }
        )
    res = run_bass_kernel_spmd(
        nc, in_maps, core_ids=list(range(N_CORES)), **run_kwargs
    )
    out = np.concatenate(
        [np.asarray(res.results[c]["out"]) for c in range(N_CORES)], axis=0
    ).astype(np.float32)
    kernel._last_results = res
    return out


# revision 9
# speedup vs baseline: 1.1050x; 1.1050x over previous
"""Bass/Trainium2 kernel for nn_BayesConv2dMF (per-sample-weight 3x3 conv).

Contract: kernel(**inputs) takes FULL unsharded inputs
  input      [32, 128, 56, 56] f32
  eps        [32, 128, 128, 3, 3] f32
  weight_psi [128, 128, 3, 3] f32
  weight_mu  [128, 128, 3, 3] f32
and returns the FULL output [32, 128, 56, 56] f32.

Strategy: data-parallel over batch across 8 NeuronCores (4 images/core).
Host prep: psi/mu are fed pre-transposed as [CI, K9, CO] f32 so the device
can DMA them in per-tap-group slices (the natural [CO,CI,3,3] layout has
taps innermost, which makes group slices 6-byte-granular). All math
(exp, eps*exp(psi)+mu, conv) stays on device.

Per image on-core (software-pipelined one image ahead):
  eps -> SBUF via SWDGE cast-DMA f32->bf16 (natural [CO, CI*9] layout)
  per tap: transpose eps -> [CI, k, CO]. Image 0 (ramp-critical) uses PE
      transposes pipelined per tap-group; steady-state images use the DMA
      XBAR transpose (dma_start_transpose, spread over the SP/Act/DVE
      rings) so the PE does nothing but conv matmuls.
  DVE: wT = epsT * exp(psiT) then wT += muT  -> [CI, K9, CO] bf16
  x   -> plain [CI, 56, 56] bf16 tile via SWDGE cast-DMA (fully contiguous
      per partition -> full-rate; no padding needed at all)
  conv: chunks of 7 output rows; taps outer so one weight load feeds the
      live chunks; 9 PSUM-accumulating matmuls per chunk (K=CI=128, bf16).
      Edge handling: tap (1,1) goes first with start=True over the full
      chunk; kw!=1 taps restrict output columns and kh!=1 taps restrict
      rows on the top/bottom chunks (out-of-range x reads are implicit
      zeros that are simply never accumulated).
  PSUM -> SBUF bf16 (ScalarE) -> DRAM bf16 (SP HWDGE); host upcasts.
  A HAM warm-up burst keeps the PE clock ungated through the input ramp.
"""

import numpy as np

import concourse.bass as bass
import concourse.tile as tile
from concourse import bacc, mybir
from concourse.bass_utils import run_bass_kernel_spmd
from concourse.masks import make_identity

B, CO, CI, KH, KW, H, W = 32, 128, 128, 3, 3, 56, 56
K9 = KH * KW
N_CORES = 8
BPC = B // N_CORES  # images per core
RB = 7  # output rows per PSUM chunk
NCHUNK = H // RB  # 8 chunks per image
F32 = mybir.dt.float32
BF16 = mybir.dt.bfloat16

# tap-group order: g1 (taps 3,4,5) first so tap 4 = (kh=1,kw=1) leads
GROUPS = [1, 0, 2]
# within-chunk tap order: group g1 first, tap (kh,1) first inside each group
TAP_ORDER = [4, 3, 5, 1, 0, 2, 7, 6, 8]

N_WARM = 26  # HAM warm-up matmuls (must fit in the pre-conv PE idle window)

NPSO = 5  # rolling PSUM chunk slots (PSUM is 8 banks: 5 + 2 pswt + 1 warm)

# image-0 x row pieces (prefix loads so early conv parts can start)
X0_BOUNDS = [0, 22, 43, H]


def tap_ranges(k):
    """Output-column range and x-column range for tap k (W-edge handling)."""
    kh, kw = divmod(k, KW)
    if kw == 0:
        return kh, 1, W, 0, W - 1  # out cols 1..55 <- x cols 0..54
    if kw == 2:
        return kh, 0, W - 1, 1, W  # out cols 0..54 <- x cols 1..55
    return kh, 0, W, 0, W  # full


def emit(nc, tc, ctx, x_d, eps_d, psit_d, mut_d, out_d):
    const = ctx.enter_context(tc.tile_pool(name="const", bufs=1))
    wpool = ctx.enter_context(tc.tile_pool(name="wpool", bufs=2))
    opool = ctx.enter_context(tc.tile_pool(name="opool", bufs=2))
    psw = ctx.enter_context(tc.tile_pool(name="psw", bufs=1, space="PSUM"))
    pso = ctx.enter_context(tc.tile_pool(name="pso", bufs=1, space="PSUM"))

    ident = const.tile([128, 128], BF16)
    make_identity(nc, ident)

    # HAM warm-up: dummy matmuls fill the pre-conv PE idle window so the
    # activity monitor releases the clock gate before the real stream.
    warm_ps = psw.tile([128, 64], F32, tag="warm", name="warm_ps", bufs=1)
    for _ in range(N_WARM):
        nc.tensor.matmul(warm_ps, ident, ident[:, :64], start=True, stop=True)

    # shared weights, host-pre-transposed to [CI, K9, CO]
    psi_t = const.tile([CI, K9, CO], F32)
    mu_t = const.tile([CI, K9, CO], F32)
    exp_psi = const.tile([CI, K9, CO], BF16)
    mu_bf = const.tile([CI, K9, CO], BF16)
    for g in GROUPS:
        sl = slice(3 * g, 3 * g + 3)
        nc.sync.dma_start(psi_t[:, sl, :], psit_d[:, sl, :])
        nc.sync.dma_start(mu_t[:, sl, :], mut_d[:, sl, :])
    for g in GROUPS:
        sl = slice(3 * g, 3 * g + 3)
        nc.scalar.activation(
            exp_psi[:, sl, :], psi_t[:, sl, :], mybir.ActivationFunctionType.Exp
        )
    # one-time bf16 cast of muT for the steady-state DVE adds (image 0 uses
    # the f32 muT directly so its chain doesn't wait for this)
    nc.scalar.copy(mu_bf, mu_t)

    NXP = 3
    xts = [const.tile([CI, H, W], BF16, name=f"xt{i}", tag=f"xt{i}") for i in range(NXP)]

    wTs = {}
    out_sbs = {}
    eps_ts = {}

    def prep_load(b):
        # cast-DMA eps and x for image b (issued ~2 images ahead so the
        # weight transposes never make a sequencer wait on them)
        eps_t = wpool.tile([CO, CI, K9], BF16, tag="eps", name=f"eps{b}")
        nc.gpsimd.dma_start(
            eps_t, eps_d[b].rearrange("co ci kh kw -> co ci (kh kw)")
        )
        eps_ts[b] = eps_t
        xt = xts[b % NXP]
        bounds = X0_BOUNDS if b == 0 else [0, H]
        for lo, hi in zip(bounds[:-1], bounds[1:]):
            nc.gpsimd.dma_start(xt[:, lo:hi, :], x_d[b][:, lo:hi, :])
        out_sbs[b] = opool.tile([CO, H, W], BF16, tag="osb", name=f"osb{b}")

    def prep_w(b):
        # per-sample weights: transpose eps taps, then wT = epsT*exp(psiT)+muT
        eps_t = eps_ts[b]
        wT = wpool.tile([CI, K9, CO], BF16, tag="wT", name=f"wT{b}")
        if b == 0:
            # ramp-critical: PE transposes, pipelined per 3-tap group
            psum_wt = psw.tile([CI, K9, CO], BF16, tag="pswt", name="pswt0", bufs=1)
            for g in GROUPS:
                sl = slice(3 * g, 3 * g + 3)
                for k in range(3 * g, 3 * g + 3):
                    nc.tensor.transpose(psum_wt[:, k, :], eps_t[:, :, k], ident)
                nc.vector.tensor_mul(
                    wT[:, sl, :], psum_wt[:, sl, :], exp_psi[:, sl, :]
                )
                nc.vector.tensor_add(wT[:, sl, :], wT[:, sl, :], mu_t[:, sl, :])
        else:
            # steady state: DMA XBAR transposes keep the PE conv-only.
            # The XBAR needs a contiguous input row, so first repack
            # [CO, CI, K9] -> [CO, K9, CI] on DVE (free-dim permutation),
            # then transpose per tap. All on the SP ring: stores tolerate
            # the sequencer time; Act's PSUM-freeing evacs stay prompt.
            eps_r = wpool.tile([CO, K9, CI], BF16, tag="epsr", name=f"epsr{b}")
            nc.vector.tensor_copy(
                eps_r, eps_t.rearrange("co ci k -> co k ci")
            )
            epsT = wpool.tile([CI, K9, CO], BF16, tag="epsT", name=f"epsT{b}")
            for k in range(K9):
                nc.sync.dma_start_transpose(epsT[:, k, :], eps_r[:, k, :])
            nc.vector.tensor_mul(wT, epsT, exp_psi)
            nc.vector.tensor_add(wT, wT, mu_bf)
        wTs[b] = wT

    slot_counter = [0]

    def conv_part(b, r0, nch):
        xt = xts[b % NXP]
        wT = wTs[b]
        out_sb = out_sbs[b]
        rows = nch * RB
        pss = []
        for c in range(nch):
            s = slot_counter[0] % NPSO
            slot_counter[0] += 1
            ps = pso.tile([CO, RB, W], F32, tag=f"pso{s}", name=f"ps_{b}_{r0}_{c}")
            pss.append(ps)
        # taps outer: one weight load per tap feeds all live chunk matmuls
        for i, k in enumerate(TAP_ORDER):
            kh, olo, ohi, xlo, xhi = tap_ranges(k)
            for c in range(nch):
                arow = r0 + c * RB
                rlo = 1 if (arow == 0 and kh == 0) else 0
                rhi = RB - (1 if (arow + RB == H and kh == 2) else 0)
                xr = arow + rlo + kh - 1
                nc.tensor.matmul(
                    pss[c][:, rlo:rhi, olo:ohi],
                    wT[:, k, :],
                    xt[:, xr : xr + rhi - rlo, xlo:xhi],
                    start=(i == 0),
                    stop=(i == K9 - 1),
                )
        for c in range(nch):
            dst = out_sb[:, r0 + c * RB : r0 + (c + 1) * RB, :]
            nc.scalar.copy(dst, pss[c])
        nc.sync.dma_start(
            out_d[b][:, r0 : r0 + rows, :], out_sb[:, r0 : r0 + rows, :]
        )

    # software-pipelined emission: loads run two images ahead, weight
    # transposes one image ahead (with eps already resident), interleaved
    # between the conv parts of the current image.
    # The final image tapers to 1-chunk parts so the drain tail shrinks.
    prep_load(0)
    prep_w(0)
    prep_load(1)
    for b in range(BPC):
        if b == 0:
            conv_part(b, 0, 3)
            conv_part(b, 3 * RB, 3)
            prep_w(1)
            conv_part(b, 6 * RB, 2)
            prep_load(2)
        elif b + 1 < BPC:
            conv_part(b, 0, 2)
            conv_part(b, 2 * RB, 2)
            prep_w(b + 1)
            conv_part(b, 4 * RB, 2)
            if b + 2 < BPC:
                prep_load(b + 2)
            conv_part(b, 6 * RB, 2)
        else:
            conv_part(b, 0, 2)
            conv_part(b, 2 * RB, 2)
            conv_part(b, 4 * RB, 2)
            conv_part(b, 6 * RB, 1)
            conv_part(b, 7 * RB, 1)


def build():
    from contextlib import ExitStack

    nc = bacc.Bacc("TRN2", target_bir_lowering=False, debug=False, num_devices=N_CORES)
    x_d = nc.dram_tensor("input", [BPC, CI, H, W], F32, kind="ExternalInput").ap()
    eps_d = nc.dram_tensor(
        "eps", [BPC, CO, CI, KH, KW], F32, kind="ExternalInput"
    ).ap()
    psit_d = nc.dram_tensor("psi_t", [CI, K9, CO], F32, kind="ExternalInput").ap()
    mut_d = nc.dram_tensor("mu_t", [CI, K9, CO], F32, kind="ExternalInput").ap()
    out_d = nc.dram_tensor("out", [BPC, CO, H, W], BF16, kind="ExternalOutput").ap()

    with tile.TileContext(nc) as tc:
        with ExitStack() as ctx:
            emit(nc, tc, ctx, x_d, eps_d, psit_d, mut_d, out_d)
    nc.compile()
    return nc


_NC_CACHE = None


def kernel(input, eps, weight_psi, weight_mu, **run_kwargs):
    global _NC_CACHE
    if _NC_CACHE is None:
        _NC_CACHE = build()
    nc = _NC_CACHE
    # host layout prep of the replicated weights: [CO,CI,KH,KW] -> [CI,K9,CO]
    psi_t = np.ascontiguousarray(
        weight_psi.reshape(CO, CI, K9).transpose(1, 2, 0), dtype=np.float32
    )
    mu_t = np.ascontiguousarray(
        weight_mu.reshape(CO, CI, K9).transpose(1, 2, 0), dtype=np.float32
    )
    in_maps = []
    for c in range(N_CORES):
        sl = slice(c * BPC, (c + 1) * BPC)
        in_maps.append(
            {
                "input": np.ascontiguousarray(input[sl], dtype=np.float32),
                "eps": np.ascontiguousarray(eps[sl], dtype=np.float32),
                "psi_t": psi_t,
                "mu_t": mu_t,
            }
        )
    res = run_bass_kernel_spmd(
        nc, in_maps, core_ids=list(range(N_CORES)), **run_kwargs
    )
    out = np.concatenate(
        [np.asarray(res.results[c]["out"]) for c in range(N_CORES)], axis=0
    ).astype(np.float32)
    kernel._last_results = res
    return out


# revision 10
# speedup vs baseline: 1.4475x; 1.3099x over previous
"""Bass/Trainium2 kernel for nn_BayesConv2dMF (per-sample-weight 3x3 conv).

Contract: kernel(**inputs) takes FULL unsharded inputs
  input      [32, 128, 56, 56] f32
  eps        [32, 128, 128, 3, 3] f32
  weight_psi [128, 128, 3, 3] f32
  weight_mu  [128, 128, 3, 3] f32
and returns the FULL output [32, 128, 56, 56] f32.

Strategy: data-parallel over batch across 8 NeuronCores (4 images/core).
Host prep: psi/mu are fed pre-transposed as [CI, K9, CO] f32 so the device
can DMA them in per-tap-group slices (the natural [CO,CI,3,3] layout has
taps innermost, which makes group slices 6-byte-granular). All math
(exp, eps*exp(psi)+mu, conv) stays on device.

Per image on-core (software-pipelined one image ahead):
  eps -> SBUF via SWDGE cast-DMA f32->bf16 (natural [CO, CI*9] layout)
  per tap: transpose eps -> [CI, k, CO]. Image 0 (ramp-critical) uses PE
      transposes pipelined per tap-group; steady-state images use the DMA
      XBAR transpose (dma_start_transpose, spread over the SP/Act/DVE
      rings) so the PE does nothing but conv matmuls.
  DVE: wT = epsT * exp(psiT) then wT += muT  -> [CI, K9, CO] bf16
  x   -> plain [CI, 56, 56] bf16 tile via SWDGE cast-DMA (fully contiguous
      per partition -> full-rate; no padding needed at all)
  conv: chunks of 7 output rows; taps outer so one weight load feeds the
      live chunks; 9 PSUM-accumulating matmuls per chunk (K=CI=128, bf16).
      Edge handling: tap (1,1) goes first with start=True over the full
      chunk; kw!=1 taps restrict output columns and kh!=1 taps restrict
      rows on the top/bottom chunks (out-of-range x reads are implicit
      zeros that are simply never accumulated).
  PSUM -> SBUF bf16 (ScalarE) -> DRAM bf16 (SP HWDGE); host upcasts.
  A HAM warm-up burst keeps the PE clock ungated through the input ramp.
"""

import numpy as np

import concourse.bass as bass
import concourse.tile as tile
from concourse import bacc, mybir
from concourse.bass_utils import run_bass_kernel_spmd
from concourse.masks import make_identity

B, CO, CI, KH, KW, H, W = 32, 128, 128, 3, 3, 56, 56
K9 = KH * KW
N_CORES = 8
BPC = B // N_CORES  # images per core
RB = 7  # output rows per PSUM chunk
NCHUNK = H // RB  # 8 chunks per image
F32 = mybir.dt.float32
BF16 = mybir.dt.bfloat16

# tap-group order: g1 (taps 3,4,5) first so tap 4 = (kh=1,kw=1) leads
GROUPS = [1, 0, 2]
# within-chunk tap order: group g1 first, tap (kh,1) first inside each group
TAP_ORDER = [4, 3, 5, 1, 0, 2, 7, 6, 8]

N_WARM = 26  # HAM warm-up matmuls (must fit in the pre-conv PE idle window)

NPSO = 5  # rolling PSUM chunk slots (PSUM is 8 banks: 5 + 2 pswt + 1 warm)

# image-0 x row pieces (prefix loads so early conv parts can start)
X0_BOUNDS = [0, 22, 43, H]


def tap_ranges(k):
    """Output-column range and x-column range for tap k (W-edge handling)."""
    kh, kw = divmod(k, KW)
    if kw == 0:
        return kh, 1, W, 0, W - 1  # out cols 1..55 <- x cols 0..54
    if kw == 2:
        return kh, 0, W - 1, 1, W  # out cols 0..54 <- x cols 1..55
    return kh, 0, W, 0, W  # full


def emit(nc, tc, ctx, x_d, eps_d, psit_d, mut_d, out_d):
    const = ctx.enter_context(tc.tile_pool(name="const", bufs=1))
    wpool = ctx.enter_context(tc.tile_pool(name="wpool", bufs=2))
    opool = ctx.enter_context(tc.tile_pool(name="opool", bufs=2))
    psw = ctx.enter_context(tc.tile_pool(name="psw", bufs=1, space="PSUM"))
    pso = ctx.enter_context(tc.tile_pool(name="pso", bufs=1, space="PSUM"))

    ident = const.tile([128, 128], BF16)
    make_identity(nc, ident)

    # HAM warm-up: dummy matmuls fill the pre-conv PE idle window so the
    # activity monitor releases the clock gate before the real stream.
    warm_ps = psw.tile([128, 64], F32, tag="warm", name="warm_ps", bufs=1)
    for _ in range(N_WARM):
        nc.tensor.matmul(warm_ps, ident, ident[:, :64], start=True, stop=True)

    # shared weights, host-pre-transposed to [CI, K9, CO]
    psi_t = const.tile([CI, K9, CO], F32)
    mu_t = const.tile([CI, K9, CO], F32)
    exp_psi = const.tile([CI, K9, CO], BF16)
    mu_bf = const.tile([CI, K9, CO], BF16)
    for g in GROUPS:
        sl = slice(3 * g, 3 * g + 3)
        nc.sync.dma_start(psi_t[:, sl, :], psit_d[:, sl, :])
        nc.sync.dma_start(mu_t[:, sl, :], mut_d[:, sl, :])
    for g in GROUPS:
        sl = slice(3 * g, 3 * g + 3)
        nc.scalar.activation(
            exp_psi[:, sl, :], psi_t[:, sl, :], mybir.ActivationFunctionType.Exp
        )
    # one-time bf16 cast of muT for the steady-state DVE adds (image 0 uses
    # the f32 muT directly so its chain doesn't wait for this)
    nc.scalar.copy(mu_bf, mu_t)

    NXP = 3
    xts = [const.tile([CI, H, W], BF16, name=f"xt{i}", tag=f"xt{i}") for i in range(NXP)]

    wTs = {}
    out_sbs = {}
    eps_ts = {}

    def prep_load(b):
        # cast-DMA eps and x for image b (issued ~2 images ahead so the
        # weight transposes never make a sequencer wait on them)
        eps_t = wpool.tile([CO, CI, K9], BF16, tag="eps", name=f"eps{b}")
        nc.gpsimd.dma_start(
            eps_t, eps_d[b].rearrange("co ci kh kw -> co ci (kh kw)")
        )
        eps_ts[b] = eps_t
        xt = xts[b % NXP]
        bounds = X0_BOUNDS if b == 0 else [0, H]
        for lo, hi in zip(bounds[:-1], bounds[1:]):
            nc.gpsimd.dma_start(xt[:, lo:hi, :], x_d[b][:, lo:hi, :])
        out_sbs[b] = opool.tile([CO, H, W], BF16, tag="osb", name=f"osb{b}")

    def prep_w(b):
        # per-sample weights: transpose eps taps, then wT = epsT*exp(psiT)+muT
        eps_t = eps_ts[b]
        wT = wpool.tile([CI, K9, CO], BF16, tag="wT", name=f"wT{b}")
        if b == 0:
            # ramp-critical: PE transposes, pipelined per 3-tap group
            psum_wt = psw.tile([CI, K9, CO], BF16, tag="pswt", name="pswt0", bufs=1)
            for g in GROUPS:
                sl = slice(3 * g, 3 * g + 3)
                for k in range(3 * g, 3 * g + 3):
                    nc.tensor.transpose(psum_wt[:, k, :], eps_t[:, :, k], ident)
                nc.vector.tensor_mul(
                    wT[:, sl, :], psum_wt[:, sl, :], exp_psi[:, sl, :]
                )
                nc.vector.tensor_add(wT[:, sl, :], wT[:, sl, :], mu_t[:, sl, :])
        else:
            psum_wt = psw.tile(
                [CI, K9, CO], BF16, tag="pswt", name=f"pswt{b}", bufs=1
            )
            for k in range(K9):
                nc.tensor.transpose(psum_wt[:, k, :], eps_t[:, :, k], ident)
            nc.vector.tensor_mul(wT, psum_wt, exp_psi)
            nc.vector.tensor_add(wT, wT, mu_bf)
        wTs[b] = wT

    slot_counter = [0]

    def conv_part(b, r0, nch):
        xt = xts[b % NXP]
        wT = wTs[b]
        out_sb = out_sbs[b]
        rows = nch * RB
        pss = []
        for c in range(nch):
            s = slot_counter[0] % NPSO
            slot_counter[0] += 1
            ps = pso.tile([CO, RB, W], F32, tag=f"pso{s}", name=f"ps_{b}_{r0}_{c}")
            pss.append(ps)
        # taps outer: one weight load per tap feeds all live chunk matmuls
        for i, k in enumerate(TAP_ORDER):
            kh, olo, ohi, xlo, xhi = tap_ranges(k)
            for c in range(nch):
                arow = r0 + c * RB
                rlo = 1 if (arow == 0 and kh == 0) else 0
                rhi = RB - (1 if (arow + RB == H and kh == 2) else 0)
                xr = arow + rlo + kh - 1
                nc.tensor.matmul(
                    pss[c][:, rlo:rhi, olo:ohi],
                    wT[:, k, :],
                    xt[:, xr : xr + rhi - rlo, xlo:xhi],
                    start=(i == 0),
                    stop=(i == K9 - 1),
                )
        for c in range(nch):
            dst = out_sb[:, r0 + c * RB : r0 + (c + 1) * RB, :]
            nc.scalar.copy(dst, pss[c])
        nc.sync.dma_start(
            out_d[b][:, r0 : r0 + rows, :], out_sb[:, r0 : r0 + rows, :]
        )

    # software-pipelined emission: loads run two images ahead, weight
    # transposes one image ahead (with eps already resident), interleaved
    # between the conv parts of the current image.
    # The final image tapers to 1-chunk parts so the drain tail shrinks.
    prep_load(0)
    prep_w(0)
    prep_load(1)
    for b in range(BPC):
        if b == 0:
            conv_part(b, 0, 3)
            conv_part(b, 3 * RB, 3)
            prep_w(1)
            conv_part(b, 6 * RB, 2)
            prep_load(2)
        elif b + 1 < BPC:
            conv_part(b, 0, 2)
            conv_part(b, 2 * RB, 2)
            prep_w(b + 1)
            conv_part(b, 4 * RB, 2)
            if b + 2 < BPC:
                prep_load(b + 2)
            conv_part(b, 6 * RB, 2)
        else:
            conv_part(b, 0, 2)
            conv_part(b, 2 * RB, 2)
            conv_part(b, 4 * RB, 2)
            conv_part(b, 6 * RB, 1)
            conv_part(b, 7 * RB, 1)


def build():
    from contextlib import ExitStack

    nc = bacc.Bacc("TRN2", target_bir_lowering=False, debug=False, num_devices=N_CORES)
    x_d = nc.dram_tensor("input", [BPC, CI, H, W], F32, kind="ExternalInput").ap()
    eps_d = nc.dram_tensor(
        "eps", [BPC, CO, CI, KH, KW], F32, kind="ExternalInput"
    ).ap()
    psit_d = nc.dram_tensor("psi_t", [CI, K9, CO], F32, kind="ExternalInput").ap()
    mut_d = nc.dram_tensor("mu_t", [CI, K9, CO], F32, kind="ExternalInput").ap()
    out_d = nc.dram_tensor("out", [BPC, CO, H, W], BF16, kind="ExternalOutput").ap()

    with tile.TileContext(nc) as tc:
        with ExitStack() as ctx:
            emit(nc, tc, ctx, x_d, eps_d, psit_d, mut_d, out_d)
    nc.compile()
    return nc


_NC_CACHE = None


def kernel(input, eps, weight_psi, weight_mu, **run_kwargs):
    global _NC_CACHE
    if _NC_CACHE is None:
        _NC_CACHE = build()
    nc = _NC_CACHE
    # host layout prep of the replicated weights: [CO,CI,KH,KW] -> [CI,K9,CO]
    psi_t = np.ascontiguousarray(
        weight_psi.reshape(CO, CI, K9).transpose(1, 2, 0), dtype=np.float32
    )
    mu_t = np.ascontiguousarray(
        weight_mu.reshape(CO, CI, K9).transpose(1, 2, 0), dtype=np.float32
    )
    in_maps = []
    for c in range(N_CORES):
        sl = slice(c * BPC, (c + 1) * BPC)
        in_maps.append(
            {
                "input": np.ascontiguousarray(input[sl], dtype=np.float32),
                "eps": np.ascontiguousarray(eps[sl], dtype=np.float32),
                "psi_t": psi_t,
                "mu_t": mu_t,
            }
        )
    res = run_bass_kernel_spmd(
        nc, in_maps, core_ids=list(range(N_CORES)), **run_kwargs
    )
    out = np.concatenate(
        [np.asarray(res.results[c]["out"]) for c in range(N_CORES)], axis=0
    ).astype(np.float32)
    kernel._last_results = res
    return out


# revision 12
# speedup vs baseline: 1.4595x; 1.0083x over previous
"""Bass/Trainium2 kernel for nn_BayesConv2dMF (per-sample-weight 3x3 conv).

Contract: kernel(**inputs) takes FULL unsharded inputs
  input      [32, 128, 56, 56] f32
  eps        [32, 128, 128, 3, 3] f32
  weight_psi [128, 128, 3, 3] f32
  weight_mu  [128, 128, 3, 3] f32
and returns the FULL output [32, 128, 56, 56] f32.

Strategy: data-parallel over batch across 8 NeuronCores (4 images/core).
Host prep: psi/mu are fed pre-transposed as [CI, K9, CO] f32 so the device
can DMA them in per-tap-group slices (the natural [CO,CI,3,3] layout has
taps innermost, which makes group slices 6-byte-granular). All math
(exp, eps*exp(psi)+mu, conv) stays on device.

Per image on-core (software-pipelined one image ahead):
  eps -> SBUF via SWDGE cast-DMA f32->bf16 (natural [CO, CI*9] layout)
  per tap: transpose eps -> [CI, k, CO]. Image 0 (ramp-critical) uses PE
      transposes pipelined per tap-group; steady-state images use the DMA
      XBAR transpose (dma_start_transpose, spread over the SP/Act/DVE
      rings) so the PE does nothing but conv matmuls.
  DVE: wT = epsT * exp(psiT) then wT += muT  -> [CI, K9, CO] bf16
  x   -> plain [CI, 56, 56] bf16 tile via SWDGE cast-DMA (fully contiguous
      per partition -> full-rate; no padding needed at all)
  conv: chunks of 7 output rows; taps outer so one weight load feeds the
      live chunks; 9 PSUM-accumulating matmuls per chunk (K=CI=128, bf16).
      Edge handling: tap (1,1) goes first with start=True over the full
      chunk; kw!=1 taps restrict output columns and kh!=1 taps restrict
      rows on the top/bottom chunks (out-of-range x reads are implicit
      zeros that are simply never accumulated).
  PSUM -> SBUF bf16 (ScalarE) -> DRAM bf16 (SP HWDGE); host upcasts.
  A HAM warm-up burst keeps the PE clock ungated through the input ramp.
"""

import numpy as np

import concourse.bass as bass
import concourse.tile as tile
from concourse import bacc, mybir
from concourse.bass_utils import run_bass_kernel_spmd
from concourse.masks import make_identity

B, CO, CI, KH, KW, H, W = 32, 128, 128, 3, 3, 56, 56
K9 = KH * KW
N_CORES = 8
BPC = B // N_CORES  # images per core
RB = 7  # output rows per PSUM chunk
NCHUNK = H // RB  # 8 chunks per image
F32 = mybir.dt.float32
F16 = mybir.dt.float16
BF16 = mybir.dt.bfloat16

# tap-group order: g1 (taps 3,4,5) first so tap 4 = (kh=1,kw=1) leads
GROUPS = [1, 0, 2]
# within-chunk tap order: group g1 first, tap (kh,1) first inside each group
TAP_ORDER = [4, 3, 5, 1, 0, 2, 7, 6, 8]

N_WARM = 26  # HAM warm-up matmuls (must fit in the pre-conv PE idle window)

NPSO = 5  # rolling PSUM chunk slots (PSUM is 8 banks: 5 + 2 pswt + 1 warm)

# image-0 x row pieces (prefix loads so early conv parts can start)
X0_BOUNDS = [0, 22, 43, H]


def tap_ranges(k):
    """Output-column range and x-column range for tap k (W-edge handling)."""
    kh, kw = divmod(k, KW)
    if kw == 0:
        return kh, 1, W, 0, W - 1  # out cols 1..55 <- x cols 0..54
    if kw == 2:
        return kh, 0, W - 1, 1, W  # out cols 0..54 <- x cols 1..55
    return kh, 0, W, 0, W  # full


def emit(nc, tc, ctx, x_d, eps_d, psit_d, mut_d, out_d):
    const = ctx.enter_context(tc.tile_pool(name="const", bufs=1))
    wpool = ctx.enter_context(tc.tile_pool(name="wpool", bufs=2))
    opool = ctx.enter_context(tc.tile_pool(name="opool", bufs=2))
    psw = ctx.enter_context(tc.tile_pool(name="psw", bufs=1, space="PSUM"))
    pso = ctx.enter_context(tc.tile_pool(name="pso", bufs=1, space="PSUM"))

    ident = const.tile([128, 128], BF16)
    make_identity(nc, ident)

    # HAM warm-up: dummy matmuls fill the pre-conv PE idle window so the
    # activity monitor releases the clock gate before the real stream.
    warm_ps = psw.tile([128, 64], F32, tag="warm", name="warm_ps", bufs=1)
    for _ in range(N_WARM):
        nc.tensor.matmul(warm_ps, ident, ident[:, :64], start=True, stop=True)

    # shared weights, host-pre-transposed to [CI, K9, CO] (psi fp16 and mu
    # bf16 from the host: mu is consumed in bf16 anyway, and fp16 psi keeps
    # exp(psi) to ~0.2% which is negligible downstream)
    psi_t = const.tile([CI, K9, CO], F16)
    exp_psi = const.tile([CI, K9, CO], BF16)
    mu_bf = const.tile([CI, K9, CO], BF16)
    for g in GROUPS:
        sl = slice(3 * g, 3 * g + 3)
        nc.sync.dma_start(psi_t[:, sl, :], psit_d[:, sl, :])
        nc.sync.dma_start(mu_bf[:, sl, :], mut_d[:, sl, :])
    for g in GROUPS:
        sl = slice(3 * g, 3 * g + 3)
        nc.scalar.activation(
            exp_psi[:, sl, :], psi_t[:, sl, :], mybir.ActivationFunctionType.Exp
        )

    NXP = 3
    xts = [const.tile([CI, H, W], BF16, name=f"xt{i}", tag=f"xt{i}") for i in range(NXP)]

    wTs = {}
    out_sbs = {}
    eps_ts = {}

    def prep_load(b):
        # cast-DMA eps and x for image b (issued ~2 images ahead so the
        # weight transposes never make a sequencer wait on them)
        eps_t = wpool.tile([CO, CI, K9], BF16, tag="eps", name=f"eps{b}")
        nc.gpsimd.dma_start(
            eps_t, eps_d[b].rearrange("co ci kh kw -> co ci (kh kw)")
        )
        eps_ts[b] = eps_t
        xt = xts[b % NXP]
        bounds = X0_BOUNDS if b == 0 else [0, H]
        for lo, hi in zip(bounds[:-1], bounds[1:]):
            nc.gpsimd.dma_start(xt[:, lo:hi, :], x_d[b][:, lo:hi, :])
        out_sbs[b] = opool.tile([CO, H, W], BF16, tag="osb", name=f"osb{b}")

    def prep_w(b):
        # per-sample weights: transpose eps taps, then wT = epsT*exp(psiT)+muT
        eps_t = eps_ts[b]
        wT = wpool.tile([CI, K9, CO], BF16, tag="wT", name=f"wT{b}")
        if b == 0:
            # ramp-critical: PE transposes, pipelined per 3-tap group
            psum_wt = psw.tile([CI, K9, CO], BF16, tag="pswt", name="pswt0", bufs=1)
            for g in GROUPS:
                sl = slice(3 * g, 3 * g + 3)
                for k in range(3 * g, 3 * g + 3):
                    nc.tensor.transpose(psum_wt[:, k, :], eps_t[:, :, k], ident)
                nc.vector.tensor_mul(
                    wT[:, sl, :], psum_wt[:, sl, :], exp_psi[:, sl, :]
                )
                nc.vector.tensor_add(wT[:, sl, :], wT[:, sl, :], mu_bf[:, sl, :])
        else:
            psum_wt = psw.tile(
                [CI, K9, CO], BF16, tag="pswt", name=f"pswt{b}", bufs=1
            )
            for k in range(K9):
                nc.tensor.transpose(psum_wt[:, k, :], eps_t[:, :, k], ident)
            nc.vector.tensor_mul(wT, psum_wt, exp_psi)
            nc.vector.tensor_add(wT, wT, mu_bf)
        wTs[b] = wT

    slot_counter = [0]

    def conv_part(b, r0, nch):
        xt = xts[b % NXP]
        wT = wTs[b]
        out_sb = out_sbs[b]
        rows = nch * RB
        pss = []
        for c in range(nch):
            s = slot_counter[0] % NPSO
            slot_counter[0] += 1
            ps = pso.tile([CO, RB, W], F32, tag=f"pso{s}", name=f"ps_{b}_{r0}_{c}")
            pss.append(ps)
        # taps outer: one weight load per tap feeds all live chunk matmuls
        for i, k in enumerate(TAP_ORDER):
            kh, olo, ohi, xlo, xhi = tap_ranges(k)
            for c in range(nch):
                arow = r0 + c * RB
                rlo = 1 if (arow == 0 and kh == 0) else 0
                rhi = RB - (1 if (arow + RB == H and kh == 2) else 0)
                xr = arow + rlo + kh - 1
                nc.tensor.matmul(
                    pss[c][:, rlo:rhi, olo:ohi],
                    wT[:, k, :],
                    xt[:, xr : xr + rhi - rlo, xlo:xhi],
                    start=(i == 0),
                    stop=(i == K9 - 1),
                )
        for c in range(nch):
            dst = out_sb[:, r0 + c * RB : r0 + (c + 1) * RB, :]
            nc.scalar.copy(dst, pss[c])
        nc.sync.dma_start(
            out_d[b][:, r0 : r0 + rows, :], out_sb[:, r0 : r0 + rows, :]
        )

    # software-pipelined emission: loads run two images ahead, weight
    # transposes one image ahead (with eps already resident), interleaved
    # between the conv parts of the current image.
    # The final image tapers to 1-chunk parts so the drain tail shrinks.
    prep_load(0)
    prep_w(0)
    prep_load(1)
    for b in range(BPC):
        if b == 0:
            conv_part(b, 0, 3)
            conv_part(b, 3 * RB, 3)
            prep_w(1)
            conv_part(b, 6 * RB, 2)
            prep_load(2)
        elif b + 1 < BPC:
            conv_part(b, 0, 2)
            conv_part(b, 2 * RB, 2)
            prep_w(b + 1)
            conv_part(b, 4 * RB, 2)
            if b + 2 < BPC:
                prep_load(b + 2)
            conv_part(b, 6 * RB, 2)
        else:
            conv_part(b, 0, 2)
            conv_part(b, 2 * RB, 2)
            conv_part(b, 4 * RB, 2)
            conv_part(b, 6 * RB, 1)
            conv_part(b, 7 * RB, 1)


def build():
    from contextlib import ExitStack

    nc = bacc.Bacc("TRN2", target_bir_lowering=False, debug=False, num_devices=N_CORES)
    x_d = nc.dram_tensor("input", [BPC, CI, H, W], F32, kind="ExternalInput").ap()
    eps_d = nc.dram_tensor(
        "eps", [BPC, CO, CI, KH, KW], F32, kind="ExternalInput"
    ).ap()
    psit_d = nc.dram_tensor("psi_t", [CI, K9, CO], F16, kind="ExternalInput").ap()
    mut_d = nc.dram_tensor("mu_t", [CI, K9, CO], BF16, kind="ExternalInput").ap()
    out_d = nc.dram_tensor("out", [BPC, CO, H, W], BF16, kind="ExternalOutput").ap()

    with tile.TileContext(nc) as tc:
        with ExitStack() as ctx:
            emit(nc, tc, ctx, x_d, eps_d, psit_d, mut_d, out_d)
    nc.compile()
    return nc


_NC_CACHE = None


def kernel(input, eps, weight_psi, weight_mu, **run_kwargs):
    global _NC_CACHE
    if _NC_CACHE is None:
        _NC_CACHE = build()
    nc = _NC_CACHE
    # host layout prep of the replicated weights: [CO,CI,KH,KW] -> [CI,K9,CO]
    import ml_dtypes

    psi_t = np.ascontiguousarray(
        weight_psi.reshape(CO, CI, K9).transpose(1, 2, 0), dtype=np.float16
    )
    mu_t = np.ascontiguousarray(
        weight_mu.reshape(CO, CI, K9).transpose(1, 2, 0).astype(ml_dtypes.bfloat16)
    )
    in_maps = []
    for c in range(N_CORES):
        sl = slice(c * BPC, (c + 1) * BPC)
        in_maps.append(
            {
                "input": np.ascontiguousarray(input[sl], dtype=np.float32),
                "eps": np.ascontiguousarray(eps[sl], dtype=np.float32),
                "psi_t": psi_t,
                "mu_t": mu_t,
            }
        )
    res = run_bass_kernel_spmd(
        nc, in_maps, core_ids=list(range(N_CORES)), **run_kwargs
    )
    out = np.concatenate(
        [np.asarray(res.results[c]["out"]) for c in range(N_CORES)], axis=0
    ).astype(np.float32)
    kernel._last_results = res
    return out
